# revision 38
# baseline (speedup 1.0000x reference)
"""HQQ grouped GEMM (MoE routing) on 8 TRN2 NeuronCores — v9 (fp8 DoubleRow).

The matmul runs in fp8e4m3 DoubleRow perf mode (0.5 cycles/row, 256-deep
contraction per instruction).  Precision is kept at better-than-bf16
parity with a 3-term split:

    y = x_hi @ W_hi + x_lo @ W_hi + x_hi @ W_lo

where x = x_hi + x_lo and W·alpha = W_hi + W_lo are exact two-word fp8
decompositions.  alpha is a per-output-column power of two chosen on the
host so every W column sits in e4m3's normal range (HQQ group scales span
20x, which otherwise lands small-scale groups in fp8 subnormals); it is
undone at PSUM-evict time by a per-partition tensor_scalar_mul (PSUM
partition axis == output column).  Each DoubleRow instruction packs one
term for a kt-PAIR (A=W[2j], B=W[2j+1]), so a PSUM group needs
3 instrs/kt-pair x 0.5 cycles/row = 0.75x the bf16 row count.

Sharding: single-expert sections (expert, contiguous oc range),
LPT-packed over the 8 cores on token*oc load — balances to ~1/8 of total
work per core (the old pair/solo split left the solo cores ~14% heavy).

DMA: W_hi streams on the sync (SP) queue, W_lo on the scalar (ACT)
queue, x/y on the gpsimd (Pool) queue — the cost model gives each queue
its own full-rate engine set, so ~8MB of traffic hides behind ~14us of
PE time.  A dummy-matmul warmup chain keeps the PE p-state ramping
during the ~3us DMA pipeline fill.

Host does dequant + fp8 split; device is pure DMA+matmul+scaled-evict.
"""
import sys
sys.path.insert(0, '/opt/trn_rl_repo')
import numpy as np

E, IN, OUT, GS, N = 8, 1024, 2816, 64, 2048
G = IN // GS
NC_ = 8
KT = IN // 128
NJ = KT // 2     # kt pairs per contraction
P = 128
NOC = OUT // P
PSUM_FP32 = 512  # fp32 columns per PSUM bank
NWARM = 46       # PE p-state warmup matmuls (~2.5us at pstate-mid)


def _make_plan(counts):
    """Joint (PE, W-DMA) balance: every core gets exactly ~NOC oc-quanta
    (equal weight-stream traffic) whose token weights sum to ~1/NC_ of
    the total token*oc work (equal PE time).  Big experts (above-average
    token count) waterfill contiguously so x stays core-local; each
    core's remaining oc slots are backfilled from the small-expert
    sequence (their x is tiny, so spreading them is cheap).  Sections
    per core are ordered heaviest-first."""
    counts = [int(c) for c in counts]
    live = [e for e in range(E) if counts[e] > 0]
    total = sum(counts[e] for e in live) * NOC
    avg = total / (len(live) * NOC)
    bigs = sorted((e for e in live if counts[e] > avg),
                  key=lambda e: -counts[e])
    smalls = sorted((e for e in live if counts[e] <= avg),
                    key=lambda e: -counts[e])
    bigseq = [e for e in bigs for _ in range(NOC)]
    smallseq = [e for e in smalls for _ in range(NOC)]
    slots_per_core = len(live) * NOC / NC_
    percore = [dict() for _ in range(NC_)]
    bi = si = 0
    for c in range(NC_):
        load = 0
        slots = int(round(slots_per_core * (c + 1))) \
            - int(round(slots_per_core * c))
        used = 0
        tgt = total * (c + 1) / NC_ - total * c / NC_
        while bi < len(bigseq) and used < slots:
            w = counts[bigseq[bi]]
            sleft = len(smallseq) - si
            s_here = min(slots - used - 1, sleft)
            avg_s = (sum(counts[e] for e in smallseq[si:si + s_here])
                     / s_here) if s_here else 0
            if c < NC_ - 1 and load + w / 2 + avg_s * s_here > tgt:
                break
            percore[c][bigseq[bi]] = percore[c].get(bigseq[bi], 0) + 1
            load += w
            used += 1
            bi += 1
        while used < slots and si < len(smallseq):
            if c < NC_ - 1 and bi < len(bigseq) \
                    and load + counts[smallseq[si]] / 2 > tgt:
                break
            percore[c][smallseq[si]] = percore[c].get(smallseq[si], 0) + 1
            load += counts[smallseq[si]]
            used += 1
            si += 1
    # stragglers (rounding/degenerate distributions): spread one oc at a
    # time onto the core with the fewest used oc slots (bounds W traffic
    # and SBUF per core)
    for e in bigseq[bi:] + smallseq[si:]:
        c = min(range(NC_), key=lambda ci: sum(percore[ci].values()))
        percore[c][e] = percore[c].get(e, 0) + 1
    # refinement: shift single boundary ocs from the max-loaded core to
    # an adjacent (same-expert-span) lighter core while it helps
    def _load(ci):
        return sum(counts[e] * n for e, n in percore[ci].items())
    for _ in range(64):
        loads = [_load(ci) for ci in range(NC_)]
        mx = max(range(NC_), key=lambda ci: loads[ci])
        best = None
        for e, n in percore[mx].items():
            if n == 0:
                continue
            for nb in range(NC_):
                if nb == mx or percore[nb].get(e, 0) == 0:
                    continue
                if sum(percore[nb].values()) >= slots_per_core + 8:
                    continue
                nl = max(loads[mx] - counts[e], loads[nb] + counts[e])
                if nl < loads[mx] and (best is None or nl < best[0]):
                    best = (nl, e, nb)
        if best is None:
            break
        _, e, nb = best
        percore[mx][e] -= 1
        percore[nb][e] = percore[nb].get(e, 0) + 1
    plan = [[] for _ in range(NC_)]
    for e in range(E):
        o0 = 0
        for ci in range(NC_):
            n = percore[ci].get(e, 0)
            if n:
                plan[ci].append({'e': e, 'o0': o0, 'ow': n * P})
                o0 += n * P
        if counts[e] > 0:
            assert o0 == OUT, f"expert {e}: {o0}"
    for ci in range(NC_):
        plan[ci].sort(key=lambda s: -counts[s['e']] * s['ow'])
    return plan


def _token_chunks(T):
    """Split the token axis into <=512 chunks [(c0, clen)]."""
    chunks, c0 = [], 0
    while c0 < T:
        take = min(T - c0, PSUM_FP32)
        chunks.append((c0, take))
        c0 += take
    return chunks


def _wblocks(ow):
    """Weight layout blocks: [(o0, bw, coloff)] — block holds cols
    [o0, o0+bw) of every kt, packed contiguously at coloff.  Graded
    sizes: small first block so the PE starts early."""
    sizes = []
    rem = ow
    if rem > 128:
        sizes.append(128)
        rem -= 128
    while rem > 256:
        sizes.append(256)
        rem -= 256
    if rem:
        sizes.append(rem)
    wbs = []
    o0 = 0
    coloff = 0
    for bw in sizes:
        wbs.append((o0, bw, coloff))
        o0 += bw
        coloff += KT * bw
    return wbs


def _build_prog(key, reps=1):
    """One Bass program of independent sections; key = ((T, ow), ...).
    Per section s (SBUF-native layouts, fp8e4m3):
      xp{s}  [128, 2*KT*T]  xp[p, h*KT*T + kt*T + t] = xh/xl[tok t, kt*128+p]
      Whi{s} [128, KT*ow]   block layout, see _wblocks
      Wlo{s} [128, KT*ow]   same layout (fp8 residual of alpha-scaled W)
      ia{s}  [128, 1] f32   ia[p, 0] = 1/alpha[p]  (per-partition)
      yp{s}  [128, noc*T]   yp[p, oc*T + t] = y[tok t, o0+oc*128+p] bf16

    PSUM groups pack gsz = max(1, 512 // T) consecutive ocs so
    small-token sections amortize the PSUM-evict instruction cost.
    """
    import concourse.mybir as mybir
    from concourse import bacc
    from concourse.tile import TileContext

    nc = bacc.Bacc('TRN2')
    dt = mybir.dt
    secs = []
    for si, (T, ow) in enumerate(key):
        noc = ow // P
        secs.append({
            'ow': ow, 'T': T, 'noc': noc,
            'chunks': _token_chunks(T), 'wbs': _wblocks(ow),
            'xp': nc.dram_tensor(f"xp{si}", [P, 2 * KT * T], dt.float8e4,
                                 kind="ExternalInput"),
            'Whi': nc.dram_tensor(f"Whi{si}", [P, KT * ow],
                                  dt.float8e4, kind="ExternalInput"),
            'Wlo': nc.dram_tensor(f"Wlo{si}", [P, KT * ow],
                                  dt.float8e4, kind="ExternalInput"),
            'ia': nc.dram_tensor(f"ia{si}", [P, 1], dt.float32,
                                 kind="ExternalInput"),
            'yp': nc.dram_tensor(f"yp{si}", [P, noc * T], dt.bfloat16,
                                 kind="ExternalOutput"),
        })

    with TileContext(nc) as tc:
        with tc.tile_pool(name="persist", bufs=1) as pp, \
             tc.tile_pool(name="psum", bufs=8, space="PSUM") as ps:

            for si, s in enumerate(secs):
                s['xall'] = pp.tile([P, 2 * KT * s['T']], dt.float8e4,
                                    name=f"xall{si}", tag=f"xall{si}")
                s['whi'] = pp.tile([P, KT * s['ow']], dt.float8e4,
                                   name=f"whi{si}", tag=f"whi{si}")
                s['wlo'] = pp.tile([P, KT * s['ow']], dt.float8e4,
                                   name=f"wlo{si}", tag=f"wlo{si}")
                s['iat'] = pp.tile([P, 1], dt.float32,
                                   name=f"iat{si}", tag=f"iat{si}")
                s['ysb'] = pp.tile([P, s['noc'] * s['T']], dt.bfloat16,
                                   name=f"ysb{si}", tag=f"ysb{si}")

            # PE p-state warmup: a no-dep DoubleRow chain on a memset
            # tile keeps the clock ramping while the input DMAs fill.
            wm = pp.tile([P, 256], dt.float8e4, name="wm", tag="wm")
            nc.vector.memset(wm[:, :], 0)
            wmp = ps.tile([P, 128], dt.float32, name="wmp", tag="pt")
            w3 = wm[:, 0:256].rearrange("p (two c) -> p two c", two=2)
            for i in range(NWARM):
                nc.tensor.matmul(wmp[:, :], w3, w3, start=(i == 0),
                                 stop=(i == NWARM - 1),
                                 perf_mode=mybir.MatmulPerfMode.DoubleRow)

            for _rep in range(reps):
                # inputs in compute order on every queue: x first (pool),
                # W blocks per section in the order the PE consumes them
                # (W_hi on sync, W_lo + inva on scalar).
                for si_, s in enumerate(secs):
                    _emit_inputs(nc, s, si_ == 0)
                est = {'gi': 0}
                for si_, s in enumerate(secs):
                    _emit_compute(nc, dt, ps, s, si_ == len(secs) - 1,
                                  si_ >= len(secs) - 2, est)
    nc.compile()
    return nc


def _emit_inputs(nc, s, split_x):
    """Queue layout (each queue transfers at full rate in parallel):
      pool   : x_hi            (term 0 consumes it first)
      scalar : x_lo, W_lo blocks (terms 1-2 run ~0.4/0.8us later)
      sync   : W_hi b0, inva, W_hi rest (+ y flushes emitted later)
    The first section's x goes in two kt-pair chunks so term 0's j-loop
    starts on pair 0-1 while pair 2-3 is still in flight."""
    T = s['T']
    H = KT * T
    xp, xall = s['xp'], s['xall']
    xcuts = ((0, 4 * T), (4 * T, H)) if split_x else ((0, H),)
    for a, b in xcuts:
        nc.gpsimd.dma_start(xall[:, a:b], xp.ap()[:, a:b])
    (o0, bw, coloff) = s['wbs'][0]
    nc.sync.dma_start(s['whi'][:, coloff:coloff + KT * bw],
                      s['Whi'].ap()[:, coloff:coloff + KT * bw])
    nc.sync.dma_start(s['iat'][:, :], s['ia'].ap()[:, :])
    for a, b in xcuts:
        nc.scalar.dma_start(xall[:, H + a:H + b], xp.ap()[:, H + a:H + b])
    for (o0, bw, coloff) in s['wbs']:
        a, b = coloff, coloff + KT * bw
        nc.scalar.dma_start(s['wlo'][:, a:b], s['Wlo'].ap()[:, a:b])
    for (o0, bw, coloff) in s['wbs'][1:]:
        a, b = coloff, coloff + KT * bw
        nc.sync.dma_start(s['whi'][:, a:b], s['Whi'].ap()[:, a:b])


def _emit_compute(nc, dt, ps, s, last_sec, tail_sec, est):
    import concourse.mybir as mybir
    chunks, wbs = s['chunks'], s['wbs']
    ow, noc, T = s['ow'], s['noc'], s['T']
    xall, whi, wlo, iat, ysb = (s['xall'], s['whi'], s['wlo'], s['iat'],
                                s['ysb'])
    H = KT * T

    def wpair(wall, j, oc):
        # DoubleRow weights AP [128, 2, 128]: [:, i, :] = cols
        # [oc*128, oc*128+128) of kt=2j+i in the block layout
        for (o0, bw, coloff) in wbs:
            if o0 <= oc * P < o0 + bw:
                a = coloff + (2 * j) * bw
                s3 = wall[:, a:a + 2 * bw].rearrange(
                    "p (two c) -> p two c", two=2)
                off = oc * P - o0
                return s3[:, :, off:off + P]
        raise AssertionError

    def xpair(h, j, c0, clen):
        # DoubleRow ifmap AP [128, 2, clen]: [:, i, :] = xh/xl tokens
        # [c0, c0+clen) of kt=2j+i
        a = h * H + 2 * j * T
        s3 = xall[:, a:a + 2 * T].rearrange("p (two c) -> p two c", two=2)
        return s3[:, :, c0:c0 + clen]

    # oc-major emission: groups complete progressively behind the weight
    # stream; y DMAs stream on the gpsimd ring behind x.  A PSUM group
    # covers gsz consecutive ocs (1 for big-token sections) with one
    # merged evict.
    total = noc * T
    gsz = max(1, PSUM_FP32 // T) if len(chunks) == 1 else 1
    ypos = 0
    yprev = 0
    oc = 0
    while oc < noc:
        g = min(gsz, noc - oc)
        if last_sec and g > 1 and oc + g == noc:
            g -= 1          # leave a lone-oc final group: minimal tail
        glast = (oc + g == noc)
        if last_sec and glast and total > PSUM_FP32 and ypos > yprev:
            # pre-final flush: everything before the final group leaves
            # on pool now, so the final sync flush waits only on the
            # final group's evict
            nc.gpsimd.dma_start(s['yp'].ap()[:, yprev:ypos],
                                ysb[:, yprev:ypos])
            yprev = ypos
        occhunks = [(c0, clen, g) for (c0, clen) in chunks]
        if glast and g == 1 and len(chunks) == 1 and chunks[0][1] > 128:
            # split the final group into token halves: the first half's
            # evict+flush overlaps the second half's matmuls, shortening
            # the end-of-kernel serial tail
            fc0, fclen = chunks[0]
            hl = fclen // 2
            occhunks = [(fc0, hl, 1), (fc0 + hl, fclen - hl, 1)]
        for (c0, clen, gg) in occhunks:
            pt = ps.tile([P, gg * clen], dt.float32, name="pt", tag="pt")
            first = True
            for k in range(gg):
                for term in range(3):
                    wall = whi if term < 2 else wlo
                    h = 1 if term == 1 else 0
                    for j in range(NJ):
                        nc.tensor.matmul(
                            pt[:, k * clen:(k + 1) * clen],
                            wpair(wall, j, oc + k),
                            xpair(h, j, c0, clen),
                            start=first,
                            stop=(k == 0 and j == NJ - 1 and term == 2),
                            skip_group_check=(k > 0),
                            perf_mode=mybir.MatmulPerfMode.DoubleRow)
                        first = False
            ydst = ysb[:, oc * T + c0:oc * T + c0 + gg * clen]
            # trailing small sections: alternate evicts between ACT and
            # DVE so the end-of-kernel evict chains don't serialize on
            # one engine right behind the last matmuls
            is_final = (last_sec and glast and (c0, clen, gg) == occhunks[-1])
            if (tail_sec and T <= 128) or is_final:
                use_act = est.get('last_ev') != 'act'
            else:
                use_act = False
            if use_act:
                nc.scalar.activation(ydst, pt[:, :],
                                     mybir.ActivationFunctionType.Copy,
                                     scale=iat[:, 0:1])
                est['last_ev'] = 'act'
            else:
                nc.vector.tensor_scalar_mul(ydst, pt[:, :], iat[:, 0:1])
                est['last_ev'] = 'dve'
            ypos += gg * clen
        oc += g
        # flush y after 6 ocs, then every ~3.  Small last sections keep
        # everything for the single final sync flush (extra 500ns-min
        # pool DMAs at the end otherwise become the critical path).
        left = noc - oc
        if last_sec and total <= PSUM_FP32:
            do_flush = ypos == total
        else:
            do_flush = (ypos == total or (yprev == 0 and ypos >= 6 * T)
                        or (yprev > 0 and left > 2
                            and ypos - yprev >= 3 * T))
        if do_flush:
            q = nc.sync if (last_sec and ypos == total) else nc.gpsimd
            q.dma_start(s['yp'].ap()[:, yprev:ypos], ysb[:, yprev:ypos])
            yprev = ypos
    if yprev < total:
        nc.sync.dma_start(s['yp'].ap()[:, yprev:total], ysb[:, yprev:total])


# ---------------------------------------------------------------- host side

def _host_prep(input, tokens_per_expert, W_q, scales, zeros):
    x = np.asarray(input, dtype=np.float32)
    counts = np.asarray(tokens_per_expert, dtype=np.int32)
    Wq = np.asarray(W_q, dtype=np.int32)
    sc = np.asarray(scales, dtype=np.float32)
    zr = np.asarray(zeros, dtype=np.float32)

    Wf = (Wq.reshape(E, G, GS, OUT).astype(np.float32) - zr[:, :, None, :]) \
        * sc[:, :, None, :]
    W = Wf.reshape(E, IN, OUT)          # fp32; fp8 split happens per section
    xT = np.ascontiguousarray(x.T)      # [IN, N] fp32
    return xT, W, counts


def _core_specs(plan, counts, xT, W):
    import ml_dtypes
    f8 = ml_dtypes.float8_e4m3
    starts = np.concatenate([[0], np.cumsum(counts)]).astype(int)
    specs = []
    for sections in plan:
        key = []
        in_map = {}
        for si, p in enumerate(sections):
            e, ow, o0 = p['e'], p['ow'], p['o0']
            T = int(counts[e])
            key.append((T, ow))
            xsel = xT[:, starts[e]:starts[e] + T]      # [IN, T] fp32
            xh = xsel.astype(f8)
            xl = (xsel - xh.astype(np.float32)).astype(f8)
            xs = np.stack([xh, xl])                    # [2, IN, T]
            xp = np.ascontiguousarray(
                xs.reshape(2, KT, P, T).transpose(2, 0, 1, 3)
                .reshape(P, 2 * KT * T))
            ws = W[e][:, o0:o0 + ow]                   # [IN, ow]
            # per-PARTITION power-of-2 normalization (shared across the
            # section's ocs: column maxes concentrate, and a per-p alpha
            # lets multi-oc PSUM groups evict with one tensor_scalar)
            pmax = np.maximum(
                np.abs(ws).reshape(IN, ow // P, P).max(axis=(0, 1)), 1e-30)
            alpha_p = 2.0 ** np.floor(np.log2(224.0 / pmax))   # [P]
            alpha = np.tile(alpha_p[None, :], (ow // P, 1)).reshape(ow)
            wn = ws * alpha[None, :]
            wh = wn.astype(f8)
            wl = (wn - wh.astype(np.float32)).astype(f8)
            blocks_h, blocks_l = [], []
            for arr, blocks in ((wh, blocks_h), (wl, blocks_l)):
                wkpo = arr.reshape(KT, P, ow)
                for (b0, bw, coloff) in _wblocks(ow):
                    blk = wkpo[:, :, b0:b0 + bw]       # [kt, p, bw]
                    blocks.append(blk.transpose(1, 0, 2).reshape(P, -1))
            ia = (1.0 / alpha_p)[:, None]              # [P, 1]
            in_map[f"xp{si}"] = xp
            in_map[f"Whi{si}"] = np.ascontiguousarray(
                np.concatenate(blocks_h, axis=1))
            in_map[f"Wlo{si}"] = np.ascontiguousarray(
                np.concatenate(blocks_l, axis=1))
            in_map[f"ia{si}"] = np.ascontiguousarray(ia.astype(np.float32))
        specs.append({'key': tuple(key), 'in_map': in_map,
                      'plan': sections})
    return specs


def _assemble(plan, counts, results):
    starts = np.concatenate([[0], np.cumsum(counts)]).astype(int)
    y = np.empty((N, OUT), np.float32)
    for sections, res in zip(plan, results):
        for si, p in enumerate(sections):
            e, ow, o0 = p['e'], p['ow'], p['o0']
            noc = ow // P
            T = int(counts[e])
            yp = np.asarray(res[f"yp{si}"]).astype(np.float32)
            yT = yp.reshape(P, noc, T).transpose(1, 0, 2).reshape(ow, T)
            y[starts[e]:starts[e] + T, o0:o0 + ow] = yT.T
    return y


# ---------------------------------------------------------------- runner

def _io_spec(nc):
    import concourse.mybir as mybir
    import jax
    in_names, out_names, out_avals, zero_outs = [], [], [], []
    for alloc in nc.m.functions[0].allocations:
        if not isinstance(alloc, mybir.MemoryLocationSet):
            continue
        name = alloc.memorylocations[0].name
        if alloc.kind == "ExternalInput":
            in_names.append(name)
        elif alloc.kind == "ExternalOutput":
            out_names.append(name)
            shape = tuple(alloc.tensor_shape)
            dtype = mybir.dt.np(alloc.dtype)
            out_avals.append(jax.core.ShapedArray(shape, dtype))
            zero_outs.append(np.zeros(shape, dtype))
    return in_names, out_names, out_avals, zero_outs


def _make_jit(nc):
    import jax
    from concourse import bass2jax
    in_names, out_names, out_avals, zero_outs = _io_spec(nc)
    assert nc.dbg_addr is None
    partition_name = (nc.partition_id_tensor.name
                      if nc.partition_id_tensor else None)
    in_names = [n for n in in_names if n != partition_name]
    nparams = len(in_names)
    all_names = in_names + out_names
    if partition_name is not None:
        all_names = all_names + [partition_name]

    def _body(*args):
        operands = list(args)
        if partition_name is not None:
            operands.append(bass2jax.partition_id_tensor())
        outs = bass2jax._bass_exec_p.bind(
            *operands,
            out_avals=tuple(out_avals),
            in_names=tuple(all_names),
            out_names=tuple(out_names),
            lowering_input_output_aliases=(),
            sim_require_finite=True,
            sim_require_nnan=True,
            nc=nc,
        )
        return tuple(outs)

    donate = tuple(range(nparams, nparams + len(out_avals)))
    return (jax.jit(_body, donate_argnums=donate, keep_unused=True),
            in_names, out_names, zero_outs)


def _run_multi(specs, ncs):
    import jax
    from concourse import bass2jax
    bass2jax.install_neuronx_cc_hook()
    devices = jax.devices()
    fns = {key: _make_jit(nc) for key, nc in ncs.items()}
    pending = []
    for i, spec in enumerate(specs):
        fn, in_names, out_names, zero_outs = fns[spec['key']]
        dev = devices[i]
        args = [jax.device_put(np.asarray(spec['in_map'][n]), dev)
                for n in in_names]
        args += [jax.device_put(z, dev) for z in zero_outs]
        pending.append((fn(*args), out_names))
    return [{nm: np.asarray(o) for nm, o in zip(out_names, outs)}
            for outs, out_names in pending]


# ---------------------------------------------------------------- entry

def kernel(input, tokens_per_expert, W_q, scales, zeros):
    xT, W, counts = _host_prep(input, tokens_per_expert, W_q, scales, zeros)
    plan = _make_plan(counts)
    specs = _core_specs(plan, counts, xT, W)
    ncs = {}
    for spec in specs:
        key = spec['key']
        if key not in ncs:
            ncs[key] = _build_prog(key)
    results = _run_multi(specs, ncs)
    return _assemble(plan, counts, results)


# revision 39
# speedup vs baseline: 1.0118x; 1.0118x over previous
"""HQQ grouped GEMM (MoE routing) on 8 TRN2 NeuronCores — v9 (fp8 DoubleRow).

The matmul runs in fp8e4m3 DoubleRow perf mode (0.5 cycles/row, 256-deep
contraction per instruction).  Precision is kept at better-than-bf16
parity with a 3-term split:

    y = x_hi @ W_hi + x_lo @ W_hi + x_hi @ W_lo

where x = x_hi + x_lo and W·alpha = W_hi + W_lo are exact two-word fp8
decompositions.  alpha is a per-output-column power of two chosen on the
host so every W column sits in e4m3's normal range (HQQ group scales span
20x, which otherwise lands small-scale groups in fp8 subnormals); it is
undone at PSUM-evict time by a per-partition tensor_scalar_mul (PSUM
partition axis == output column).  Each DoubleRow instruction packs one
term for a kt-PAIR (A=W[2j], B=W[2j+1]), so a PSUM group needs
3 instrs/kt-pair x 0.5 cycles/row = 0.75x the bf16 row count.

Sharding: single-expert sections (expert, contiguous oc range),
LPT-packed over the 8 cores on token*oc load — balances to ~1/8 of total
work per core (the old pair/solo split left the solo cores ~14% heavy).

DMA: W_hi streams on the sync (SP) queue, W_lo on the scalar (ACT)
queue, x/y on the gpsimd (Pool) queue — the cost model gives each queue
its own full-rate engine set, so ~8MB of traffic hides behind ~14us of
PE time.  A dummy-matmul warmup chain keeps the PE p-state ramping
during the ~3us DMA pipeline fill.

Host does dequant + fp8 split; device is pure DMA+matmul+scaled-evict.
"""
import sys
sys.path.insert(0, '/opt/trn_rl_repo')
import numpy as np

E, IN, OUT, GS, N = 8, 1024, 2816, 64, 2048
G = IN // GS
NC_ = 8
KT = IN // 128
NJ = KT // 2     # kt pairs per contraction
P = 128
NOC = OUT // P
PSUM_FP32 = 512  # fp32 columns per PSUM bank
NWARM = 46       # PE p-state warmup matmuls (~2.5us at pstate-mid)


def _make_plan(counts):
    """Joint (PE, W-DMA) balance: every core gets exactly ~NOC oc-quanta
    (equal weight-stream traffic) whose token weights sum to ~1/NC_ of
    the total token*oc work (equal PE time).  Big experts (above-average
    token count) waterfill contiguously so x stays core-local; each
    core's remaining oc slots are backfilled from the small-expert
    sequence (their x is tiny, so spreading them is cheap).  Sections
    per core are ordered heaviest-first."""
    counts = [int(c) for c in counts]
    live = [e for e in range(E) if counts[e] > 0]
    total = sum(counts[e] for e in live) * NOC
    avg = total / (len(live) * NOC)
    bigs = sorted((e for e in live if counts[e] > avg),
                  key=lambda e: -counts[e])
    smalls = sorted((e for e in live if counts[e] <= avg),
                    key=lambda e: -counts[e])
    bigseq = [e for e in bigs for _ in range(NOC)]
    smallseq = [e for e in smalls for _ in range(NOC)]
    slots_per_core = len(live) * NOC / NC_
    percore = [dict() for _ in range(NC_)]
    bi = si = 0
    for c in range(NC_):
        load = 0
        slots = int(round(slots_per_core * (c + 1))) \
            - int(round(slots_per_core * c))
        used = 0
        tgt = total * (c + 1) / NC_ - total * c / NC_
        while bi < len(bigseq) and used < slots:
            w = counts[bigseq[bi]]
            sleft = len(smallseq) - si
            s_here = min(slots - used - 1, sleft)
            avg_s = (sum(counts[e] for e in smallseq[si:si + s_here])
                     / s_here) if s_here else 0
            if c < NC_ - 1 and load + w / 2 + avg_s * s_here > tgt:
                break
            percore[c][bigseq[bi]] = percore[c].get(bigseq[bi], 0) + 1
            load += w
            used += 1
            bi += 1
        while used < slots and si < len(smallseq):
            if c < NC_ - 1 and bi < len(bigseq) \
                    and load + counts[smallseq[si]] / 2 > tgt:
                break
            percore[c][smallseq[si]] = percore[c].get(smallseq[si], 0) + 1
            load += counts[smallseq[si]]
            used += 1
            si += 1
    # stragglers (rounding/degenerate distributions): spread one oc at a
    # time onto the core with the fewest used oc slots (bounds W traffic
    # and SBUF per core)
    for e in bigseq[bi:] + smallseq[si:]:
        c = min(range(NC_), key=lambda ci: sum(percore[ci].values()))
        percore[c][e] = percore[c].get(e, 0) + 1
    # refinement: shift single boundary ocs from the max-loaded core to
    # an adjacent (same-expert-span) lighter core while it helps
    def _load(ci):
        return sum(counts[e] * n for e, n in percore[ci].items())
    for _ in range(64):
        loads = [_load(ci) for ci in range(NC_)]
        mx = max(range(NC_), key=lambda ci: loads[ci])
        best = None
        for e, n in percore[mx].items():
            if n == 0:
                continue
            for nb in range(NC_):
                if nb == mx or percore[nb].get(e, 0) == 0:
                    continue
                if sum(percore[nb].values()) >= slots_per_core + 8:
                    continue
                nl = max(loads[mx] - counts[e], loads[nb] + counts[e])
                if nl < loads[mx] and (best is None or nl < best[0]):
                    best = (nl, e, nb)
        if best is None:
            break
        _, e, nb = best
        percore[mx][e] -= 1
        percore[nb][e] = percore[nb].get(e, 0) + 1
    plan = [[] for _ in range(NC_)]
    for e in range(E):
        o0 = 0
        for ci in range(NC_):
            n = percore[ci].get(e, 0)
            if n:
                plan[ci].append({'e': e, 'o0': o0, 'ow': n * P})
                o0 += n * P
        if counts[e] > 0:
            assert o0 == OUT, f"expert {e}: {o0}"
    for ci in range(NC_):
        plan[ci].sort(key=lambda s: -counts[s['e']] * s['ow'])
    return plan


def _token_chunks(T):
    """Split the token axis into <=512 chunks [(c0, clen)]."""
    chunks, c0 = [], 0
    while c0 < T:
        take = min(T - c0, PSUM_FP32)
        chunks.append((c0, take))
        c0 += take
    return chunks


def _wblocks(ow):
    """Weight layout blocks: [(o0, bw, coloff)] — block holds cols
    [o0, o0+bw) of every kt, packed contiguously at coloff.  Graded
    sizes: small first block so the PE starts early."""
    sizes = []
    rem = ow
    if rem > 128:
        sizes.append(128)
        rem -= 128
    while rem > 256:
        sizes.append(256)
        rem -= 256
    if rem:
        sizes.append(rem)
    wbs = []
    o0 = 0
    coloff = 0
    for bw in sizes:
        wbs.append((o0, bw, coloff))
        o0 += bw
        coloff += KT * bw
    return wbs


def _build_prog(key, reps=1):
    """One Bass program of independent sections; key = ((T, ow), ...).
    Per section s (SBUF-native layouts, fp8e4m3):
      xp{s}  [128, 2*KT*T]  xp[p, h*KT*T + kt*T + t] = xh/xl[tok t, kt*128+p]
      Whi{s} [128, KT*ow]   block layout, see _wblocks
      Wlo{s} [128, KT*ow]   same layout (fp8 residual of alpha-scaled W)
      ia{s}  [128, 1] f32   ia[p, 0] = 1/alpha[p]  (per-partition)
      yp{s}  [128, noc*T]   yp[p, oc*T + t] = y[tok t, o0+oc*128+p] bf16

    PSUM groups pack gsz = max(1, 512 // T) consecutive ocs so
    small-token sections amortize the PSUM-evict instruction cost.
    """
    import concourse.mybir as mybir
    from concourse import bacc
    from concourse.tile import TileContext

    nc = bacc.Bacc('TRN2')
    dt = mybir.dt
    secs = []
    for si, (T, ow) in enumerate(key):
        noc = ow // P
        secs.append({
            'ow': ow, 'T': T, 'noc': noc,
            'chunks': _token_chunks(T), 'wbs': _wblocks(ow),
            'xp': nc.dram_tensor(f"xp{si}", [P, 2 * KT * T], dt.float8e4,
                                 kind="ExternalInput"),
            'Whi': nc.dram_tensor(f"Whi{si}", [P, KT * ow],
                                  dt.float8e4, kind="ExternalInput"),
            'Wlo': nc.dram_tensor(f"Wlo{si}", [P, KT * ow],
                                  dt.float8e4, kind="ExternalInput"),
            'ia': nc.dram_tensor(f"ia{si}", [P, 1], dt.float32,
                                 kind="ExternalInput"),
            'yp': nc.dram_tensor(f"yp{si}", [P, noc * T], dt.bfloat16,
                                 kind="ExternalOutput"),
        })

    with TileContext(nc) as tc:
        with tc.tile_pool(name="persist", bufs=1) as pp, \
             tc.tile_pool(name="psum", bufs=8, space="PSUM") as ps:

            for si, s in enumerate(secs):
                s['xall'] = pp.tile([P, 2 * KT * s['T']], dt.float8e4,
                                    name=f"xall{si}", tag=f"xall{si}")
                s['whi'] = pp.tile([P, KT * s['ow']], dt.float8e4,
                                   name=f"whi{si}", tag=f"whi{si}")
                s['wlo'] = pp.tile([P, KT * s['ow']], dt.float8e4,
                                   name=f"wlo{si}", tag=f"wlo{si}")
                s['iat'] = pp.tile([P, 1], dt.float32,
                                   name=f"iat{si}", tag=f"iat{si}")
                s['ysb'] = pp.tile([P, s['noc'] * s['T']], dt.bfloat16,
                                   name=f"ysb{si}", tag=f"ysb{si}")

            # PE p-state warmup: a no-dep DoubleRow chain on a memset
            # tile keeps the clock ramping while the input DMAs fill.
            wm = pp.tile([P, 256], dt.float8e4, name="wm", tag="wm")
            nc.vector.memset(wm[:, :], 0)
            wmp = ps.tile([P, 128], dt.float32, name="wmp", tag="pt")
            w3 = wm[:, 0:256].rearrange("p (two c) -> p two c", two=2)
            for i in range(NWARM):
                nc.tensor.matmul(wmp[:, :], w3, w3, start=(i == 0),
                                 stop=(i == NWARM - 1),
                                 perf_mode=mybir.MatmulPerfMode.DoubleRow)

            for _rep in range(reps):
                # inputs in compute order on every queue: x first (pool),
                # W blocks per section in the order the PE consumes them
                # (W_hi on sync, W_lo + inva on scalar).
                for si_, s in enumerate(secs):
                    _emit_inputs(nc, s, si_ == 0)
                est = {'gi': 0}
                for si_, s in enumerate(secs):
                    _emit_compute(nc, dt, ps, s, si_ == len(secs) - 1,
                                  si_ >= len(secs) - 2, est)
    nc.compile()
    return nc


def _emit_inputs(nc, s, split_x):
    """Queue layout (each queue transfers at full rate in parallel):
      pool   : x_hi            (term 0 consumes it first)
      scalar : x_lo, W_lo blocks (terms 1-2 run ~0.4/0.8us later)
      sync   : W_hi b0, inva, W_hi rest (+ y flushes emitted later)
    The first section's x goes in two kt-pair chunks so term 0's j-loop
    starts on pair 0-1 while pair 2-3 is still in flight."""
    T = s['T']
    H = KT * T
    xp, xall = s['xp'], s['xall']
    xcuts = ((0, 4 * T), (4 * T, H)) if split_x else ((0, H),)
    for a, b in xcuts:
        nc.gpsimd.dma_start(xall[:, a:b], xp.ap()[:, a:b])
    (o0, bw, coloff) = s['wbs'][0]
    nc.sync.dma_start(s['whi'][:, coloff:coloff + KT * bw],
                      s['Whi'].ap()[:, coloff:coloff + KT * bw])
    nc.sync.dma_start(s['iat'][:, :], s['ia'].ap()[:, :])
    for a, b in xcuts:
        nc.scalar.dma_start(xall[:, H + a:H + b], xp.ap()[:, H + a:H + b])
    for (o0, bw, coloff) in s['wbs']:
        a, b = coloff, coloff + KT * bw
        nc.scalar.dma_start(s['wlo'][:, a:b], s['Wlo'].ap()[:, a:b])
    for (o0, bw, coloff) in s['wbs'][1:]:
        a, b = coloff, coloff + KT * bw
        nc.sync.dma_start(s['whi'][:, a:b], s['Whi'].ap()[:, a:b])


def _emit_compute(nc, dt, ps, s, last_sec, tail_sec, est):
    import concourse.mybir as mybir
    chunks, wbs = s['chunks'], s['wbs']
    ow, noc, T = s['ow'], s['noc'], s['T']
    xall, whi, wlo, iat, ysb = (s['xall'], s['whi'], s['wlo'], s['iat'],
                                s['ysb'])
    H = KT * T

    def wpair(wall, j, oc):
        # DoubleRow weights AP [128, 2, 128]: [:, i, :] = cols
        # [oc*128, oc*128+128) of kt=2j+i in the block layout
        for (o0, bw, coloff) in wbs:
            if o0 <= oc * P < o0 + bw:
                a = coloff + (2 * j) * bw
                s3 = wall[:, a:a + 2 * bw].rearrange(
                    "p (two c) -> p two c", two=2)
                off = oc * P - o0
                return s3[:, :, off:off + P]
        raise AssertionError

    def xpair(h, j, c0, clen):
        # DoubleRow ifmap AP [128, 2, clen]: [:, i, :] = xh/xl tokens
        # [c0, c0+clen) of kt=2j+i
        a = h * H + 2 * j * T
        s3 = xall[:, a:a + 2 * T].rearrange("p (two c) -> p two c", two=2)
        return s3[:, :, c0:c0 + clen]

    # oc-major emission: groups complete progressively behind the weight
    # stream; y DMAs stream on the gpsimd ring behind x.  A PSUM group
    # covers gsz consecutive ocs (1 for big-token sections) with one
    # merged evict.
    total = noc * T
    gsz = max(1, PSUM_FP32 // T) if len(chunks) == 1 else 1
    ypos = 0
    yprev = 0
    oc = 0
    while oc < noc:
        g = min(gsz, noc - oc)
        if last_sec and g > 1 and oc + g == noc:
            g -= 1          # leave a lone-oc final group: minimal tail
        glast = (oc + g == noc)
        occhunks = [(c0, clen, g) for (c0, clen) in chunks]
        if glast and g == 1 and len(chunks) == 1 and chunks[0][1] > 128:
            # split the final group into token halves: the first half's
            # evict+flush overlaps the second half's matmuls, shortening
            # the end-of-kernel serial tail
            fc0, fclen = chunks[0]
            hl = fclen // 2
            occhunks = [(fc0, hl, 1), (fc0 + hl, fclen - hl, 1)]
        for (c0, clen, gg) in occhunks:
            pt = ps.tile([P, gg * clen], dt.float32, name="pt", tag="pt")
            first = True
            for k in range(gg):
                for term in range(3):
                    wall = whi if term < 2 else wlo
                    h = 1 if term == 1 else 0
                    for j in range(NJ):
                        nc.tensor.matmul(
                            pt[:, k * clen:(k + 1) * clen],
                            wpair(wall, j, oc + k),
                            xpair(h, j, c0, clen),
                            start=first,
                            stop=(k == 0 and j == NJ - 1 and term == 2),
                            skip_group_check=(k > 0),
                            perf_mode=mybir.MatmulPerfMode.DoubleRow)
                        first = False
            ydst = ysb[:, oc * T + c0:oc * T + c0 + gg * clen]
            # trailing small sections: alternate evicts between ACT and
            # DVE so the end-of-kernel evict chains don't serialize on
            # one engine right behind the last matmuls
            is_final = (last_sec and glast and (c0, clen, gg) == occhunks[-1])
            if (tail_sec and T <= 128) or is_final:
                use_act = est.get('last_ev') != 'act'
            else:
                use_act = False
            if use_act:
                nc.scalar.activation(ydst, pt[:, :],
                                     mybir.ActivationFunctionType.Copy,
                                     scale=iat[:, 0:1])
                est['last_ev'] = 'act'
            else:
                nc.vector.tensor_scalar_mul(ydst, pt[:, :], iat[:, 0:1])
                est['last_ev'] = 'dve'
            ypos += gg * clen
        oc += g
        # flush y after 6 ocs, then every ~3.  Small last sections keep
        # everything for the single final sync flush (extra 500ns-min
        # pool DMAs at the end otherwise become the critical path).
        left = noc - oc
        if last_sec and total <= PSUM_FP32:
            do_flush = ypos == total
        else:
            do_flush = (ypos == total or (yprev == 0 and ypos >= 6 * T)
                        or (yprev > 0 and left > 2
                            and ypos - yprev >= 3 * T))
        if do_flush:
            q = nc.sync if (last_sec and ypos == total) else nc.gpsimd
            q.dma_start(s['yp'].ap()[:, yprev:ypos], ysb[:, yprev:ypos])
            yprev = ypos
    if yprev < total:
        nc.sync.dma_start(s['yp'].ap()[:, yprev:total], ysb[:, yprev:total])


# ---------------------------------------------------------------- host side

def _host_prep(input, tokens_per_expert, W_q, scales, zeros):
    x = np.asarray(input, dtype=np.float32)
    counts = np.asarray(tokens_per_expert, dtype=np.int32)
    Wq = np.asarray(W_q, dtype=np.int32)
    sc = np.asarray(scales, dtype=np.float32)
    zr = np.asarray(zeros, dtype=np.float32)

    Wf = (Wq.reshape(E, G, GS, OUT).astype(np.float32) - zr[:, :, None, :]) \
        * sc[:, :, None, :]
    W = Wf.reshape(E, IN, OUT)          # fp32; fp8 split happens per section
    xT = np.ascontiguousarray(x.T)      # [IN, N] fp32
    return xT, W, counts


def _core_specs(plan, counts, xT, W):
    import ml_dtypes
    f8 = ml_dtypes.float8_e4m3
    starts = np.concatenate([[0], np.cumsum(counts)]).astype(int)
    specs = []
    for sections in plan:
        key = []
        in_map = {}
        for si, p in enumerate(sections):
            e, ow, o0 = p['e'], p['ow'], p['o0']
            T = int(counts[e])
            key.append((T, ow))
            xsel = xT[:, starts[e]:starts[e] + T]      # [IN, T] fp32
            xh = xsel.astype(f8)
            xl = (xsel - xh.astype(np.float32)).astype(f8)
            xs = np.stack([xh, xl])                    # [2, IN, T]
            xp = np.ascontiguousarray(
                xs.reshape(2, KT, P, T).transpose(2, 0, 1, 3)
                .reshape(P, 2 * KT * T))
            ws = W[e][:, o0:o0 + ow]                   # [IN, ow]
            # per-PARTITION power-of-2 normalization (shared across the
            # section's ocs: column maxes concentrate, and a per-p alpha
            # lets multi-oc PSUM groups evict with one tensor_scalar)
            pmax = np.maximum(
                np.abs(ws).reshape(IN, ow // P, P).max(axis=(0, 1)), 1e-30)
            alpha_p = 2.0 ** np.floor(np.log2(224.0 / pmax))   # [P]
            alpha = np.tile(alpha_p[None, :], (ow // P, 1)).reshape(ow)
            wn = ws * alpha[None, :]
            wh = wn.astype(f8)
            wl = (wn - wh.astype(np.float32)).astype(f8)
            blocks_h, blocks_l = [], []
            for arr, blocks in ((wh, blocks_h), (wl, blocks_l)):
                wkpo = arr.reshape(KT, P, ow)
                for (b0, bw, coloff) in _wblocks(ow):
                    blk = wkpo[:, :, b0:b0 + bw]       # [kt, p, bw]
                    blocks.append(blk.transpose(1, 0, 2).reshape(P, -1))
            ia = (1.0 / alpha_p)[:, None]              # [P, 1]
            in_map[f"xp{si}"] = xp
            in_map[f"Whi{si}"] = np.ascontiguousarray(
                np.concatenate(blocks_h, axis=1))
            in_map[f"Wlo{si}"] = np.ascontiguousarray(
                np.concatenate(blocks_l, axis=1))
            in_map[f"ia{si}"] = np.ascontiguousarray(ia.astype(np.float32))
        specs.append({'key': tuple(key), 'in_map': in_map,
                      'plan': sections})
    return specs


def _assemble(plan, counts, results):
    starts = np.concatenate([[0], np.cumsum(counts)]).astype(int)
    y = np.empty((N, OUT), np.float32)
    for sections, res in zip(plan, results):
        for si, p in enumerate(sections):
            e, ow, o0 = p['e'], p['ow'], p['o0']
            noc = ow // P
            T = int(counts[e])
            yp = np.asarray(res[f"yp{si}"]).astype(np.float32)
            yT = yp.reshape(P, noc, T).transpose(1, 0, 2).reshape(ow, T)
            y[starts[e]:starts[e] + T, o0:o0 + ow] = yT.T
    return y


# ---------------------------------------------------------------- runner

def _io_spec(nc):
    import concourse.mybir as mybir
    import jax
    in_names, out_names, out_avals, zero_outs = [], [], [], []
    for alloc in nc.m.functions[0].allocations:
        if not isinstance(alloc, mybir.MemoryLocationSet):
            continue
        name = alloc.memorylocations[0].name
        if alloc.kind == "ExternalInput":
            in_names.append(name)
        elif alloc.kind == "ExternalOutput":
            out_names.append(name)
            shape = tuple(alloc.tensor_shape)
            dtype = mybir.dt.np(alloc.dtype)
            out_avals.append(jax.core.ShapedArray(shape, dtype))
            zero_outs.append(np.zeros(shape, dtype))
    return in_names, out_names, out_avals, zero_outs


def _make_jit(nc):
    import jax
    from concourse import bass2jax
    in_names, out_names, out_avals, zero_outs = _io_spec(nc)
    assert nc.dbg_addr is None
    partition_name = (nc.partition_id_tensor.name
                      if nc.partition_id_tensor else None)
    in_names = [n for n in in_names if n != partition_name]
    nparams = len(in_names)
    all_names = in_names + out_names
    if partition_name is not None:
        all_names = all_names + [partition_name]

    def _body(*args):
        operands = list(args)
        if partition_name is not None:
            operands.append(bass2jax.partition_id_tensor())
        outs = bass2jax._bass_exec_p.bind(
            *operands,
            out_avals=tuple(out_avals),
            in_names=tuple(all_names),
            out_names=tuple(out_names),
            lowering_input_output_aliases=(),
            sim_require_finite=True,
            sim_require_nnan=True,
            nc=nc,
        )
        return tuple(outs)

    donate = tuple(range(nparams, nparams + len(out_avals)))
    return (jax.jit(_body, donate_argnums=donate, keep_unused=True),
            in_names, out_names, zero_outs)


def _run_multi(specs, ncs):
    import jax
    from concourse import bass2jax
    bass2jax.install_neuronx_cc_hook()
    devices = jax.devices()
    fns = {key: _make_jit(nc) for key, nc in ncs.items()}
    pending = []
    for i, spec in enumerate(specs):
        fn, in_names, out_names, zero_outs = fns[spec['key']]
        dev = devices[i]
        args = [jax.device_put(np.asarray(spec['in_map'][n]), dev)
                for n in in_names]
        args += [jax.device_put(z, dev) for z in zero_outs]
        pending.append((fn(*args), out_names))
    return [{nm: np.asarray(o) for nm, o in zip(out_names, outs)}
            for outs, out_names in pending]


# ---------------------------------------------------------------- entry

def kernel(input, tokens_per_expert, W_q, scales, zeros):
    xT, W, counts = _host_prep(input, tokens_per_expert, W_q, scales, zeros)
    plan = _make_plan(counts)
    specs = _core_specs(plan, counts, xT, W)
    ncs = {}
    for spec in specs:
        key = spec['key']
        if key not in ncs:
            ncs[key] = _build_prog(key)
    results = _run_multi(specs, ncs)
    return _assemble(plan, counts, results)


# revision 40
# speedup vs baseline: 1.0173x; 1.0054x over previous
"""HQQ grouped GEMM (MoE routing) on 8 TRN2 NeuronCores — v9 (fp8 DoubleRow).

The matmul runs in fp8e4m3 DoubleRow perf mode (0.5 cycles/row, 256-deep
contraction per instruction).  Precision is kept at better-than-bf16
parity with a 3-term split:

    y = x_hi @ W_hi + x_lo @ W_hi + x_hi @ W_lo

where x = x_hi + x_lo and W·alpha = W_hi + W_lo are exact two-word fp8
decompositions.  alpha is a per-output-column power of two chosen on the
host so every W column sits in e4m3's normal range (HQQ group scales span
20x, which otherwise lands small-scale groups in fp8 subnormals); it is
undone at PSUM-evict time by a per-partition tensor_scalar_mul (PSUM
partition axis == output column).  Each DoubleRow instruction packs one
term for a kt-PAIR (A=W[2j], B=W[2j+1]), so a PSUM group needs
3 instrs/kt-pair x 0.5 cycles/row = 0.75x the bf16 row count.

Sharding: single-expert sections (expert, contiguous oc range),
LPT-packed over the 8 cores on token*oc load — balances to ~1/8 of total
work per core (the old pair/solo split left the solo cores ~14% heavy).

DMA: W_hi streams on the sync (SP) queue, W_lo on the scalar (ACT)
queue, x/y on the gpsimd (Pool) queue — the cost model gives each queue
its own full-rate engine set, so ~8MB of traffic hides behind ~14us of
PE time.  A dummy-matmul warmup chain keeps the PE p-state ramping
during the ~3us DMA pipeline fill.

Host does dequant + fp8 split; device is pure DMA+matmul+scaled-evict.
"""
import sys
sys.path.insert(0, '/opt/trn_rl_repo')
import numpy as np

E, IN, OUT, GS, N = 8, 1024, 2816, 64, 2048
G = IN // GS
NC_ = 8
KT = IN // 128
NJ = KT // 2     # kt pairs per contraction
P = 128
NOC = OUT // P
PSUM_FP32 = 512  # fp32 columns per PSUM bank
NWARM = 46       # PE p-state warmup matmuls (~2.5us at pstate-mid)


def _make_plan(counts):
    """Joint (PE, W-DMA) balance: every core gets exactly ~NOC oc-quanta
    (equal weight-stream traffic) whose token weights sum to ~1/NC_ of
    the total token*oc work (equal PE time).  Big experts (above-average
    token count) waterfill contiguously so x stays core-local; each
    core's remaining oc slots are backfilled from the small-expert
    sequence (their x is tiny, so spreading them is cheap).  Sections
    per core are ordered heaviest-first."""
    counts = [int(c) for c in counts]
    live = [e for e in range(E) if counts[e] > 0]
    total = sum(counts[e] for e in live) * NOC
    avg = total / (len(live) * NOC)
    bigs = sorted((e for e in live if counts[e] > avg),
                  key=lambda e: -counts[e])
    smalls = sorted((e for e in live if counts[e] <= avg),
                    key=lambda e: -counts[e])
    bigseq = [e for e in bigs for _ in range(NOC)]
    smallseq = [e for e in smalls for _ in range(NOC)]
    slots_per_core = len(live) * NOC / NC_
    percore = [dict() for _ in range(NC_)]
    bi = si = 0
    for c in range(NC_):
        load = 0
        slots = int(round(slots_per_core * (c + 1))) \
            - int(round(slots_per_core * c))
        used = 0
        tgt = total * (c + 1) / NC_ - total * c / NC_
        while bi < len(bigseq) and used < slots:
            w = counts[bigseq[bi]]
            sleft = len(smallseq) - si
            s_here = min(slots - used - 1, sleft)
            avg_s = (sum(counts[e] for e in smallseq[si:si + s_here])
                     / s_here) if s_here else 0
            if c < NC_ - 1 and load + w / 2 + avg_s * s_here > tgt:
                break
            percore[c][bigseq[bi]] = percore[c].get(bigseq[bi], 0) + 1
            load += w
            used += 1
            bi += 1
        while used < slots and si < len(smallseq):
            if c < NC_ - 1 and bi < len(bigseq) \
                    and load + counts[smallseq[si]] / 2 > tgt:
                break
            percore[c][smallseq[si]] = percore[c].get(smallseq[si], 0) + 1
            load += counts[smallseq[si]]
            used += 1
            si += 1
    # stragglers (rounding/degenerate distributions): spread one oc at a
    # time onto the core with the fewest used oc slots (bounds W traffic
    # and SBUF per core)
    for e in bigseq[bi:] + smallseq[si:]:
        c = min(range(NC_), key=lambda ci: sum(percore[ci].values()))
        percore[c][e] = percore[c].get(e, 0) + 1
    # refinement: shift single boundary ocs from the max-loaded core to
    # an adjacent (same-expert-span) lighter core while it helps
    def _load(ci):
        return sum(counts[e] * n for e, n in percore[ci].items())
    for _ in range(64):
        loads = [_load(ci) for ci in range(NC_)]
        mx = max(range(NC_), key=lambda ci: loads[ci])
        best = None
        for e, n in percore[mx].items():
            if n == 0:
                continue
            for nb in range(NC_):
                if nb == mx or percore[nb].get(e, 0) == 0:
                    continue
                if sum(percore[nb].values()) >= slots_per_core + 8:
                    continue
                nl = max(loads[mx] - counts[e], loads[nb] + counts[e])
                if nl < loads[mx] and (best is None or nl < best[0]):
                    best = (nl, e, nb)
        if best is None:
            break
        _, e, nb = best
        percore[mx][e] -= 1
        percore[nb][e] = percore[nb].get(e, 0) + 1
    plan = [[] for _ in range(NC_)]
    for e in range(E):
        o0 = 0
        for ci in range(NC_):
            n = percore[ci].get(e, 0)
            if n:
                plan[ci].append({'e': e, 'o0': o0, 'ow': n * P})
                o0 += n * P
        if counts[e] > 0:
            assert o0 == OUT, f"expert {e}: {o0}"
    for ci in range(NC_):
        plan[ci].sort(key=lambda s: -counts[s['e']] * s['ow'])
        # middle sections ascending: only the last two sections should
        # finish inside the end-of-kernel flush window (2 usable queues)
        if len(plan[ci]) > 3:
            plan[ci] = ([plan[ci][0]] + plan[ci][1:-1][::-1]
                        + [plan[ci][-1]])
    return plan


def _token_chunks(T):
    """Split the token axis into <=512 chunks [(c0, clen)]."""
    chunks, c0 = [], 0
    while c0 < T:
        take = min(T - c0, PSUM_FP32)
        chunks.append((c0, take))
        c0 += take
    return chunks


def _wblocks(ow):
    """Weight layout blocks: [(o0, bw, coloff)] — block holds cols
    [o0, o0+bw) of every kt, packed contiguously at coloff.  Graded
    sizes: small first block so the PE starts early."""
    sizes = []
    rem = ow
    if rem > 128:
        sizes.append(128)
        rem -= 128
    while rem > 256:
        sizes.append(256)
        rem -= 256
    if rem:
        sizes.append(rem)
    wbs = []
    o0 = 0
    coloff = 0
    for bw in sizes:
        wbs.append((o0, bw, coloff))
        o0 += bw
        coloff += KT * bw
    return wbs


def _build_prog(key, reps=1):
    """One Bass program of independent sections; key = ((T, ow), ...).
    Per section s (SBUF-native layouts, fp8e4m3):
      xp{s}  [128, 2*KT*T]  xp[p, h*KT*T + kt*T + t] = xh/xl[tok t, kt*128+p]
      Whi{s} [128, KT*ow]   block layout, see _wblocks
      Wlo{s} [128, KT*ow]   same layout (fp8 residual of alpha-scaled W)
      ia{s}  [128, 1] f32   ia[p, 0] = 1/alpha[p]  (per-partition)
      yp{s}  [128, noc*T]   yp[p, oc*T + t] = y[tok t, o0+oc*128+p] bf16

    PSUM groups pack gsz = max(1, 512 // T) consecutive ocs so
    small-token sections amortize the PSUM-evict instruction cost.
    """
    import concourse.mybir as mybir
    from concourse import bacc
    from concourse.tile import TileContext

    nc = bacc.Bacc('TRN2')
    dt = mybir.dt
    secs = []
    for si, (T, ow) in enumerate(key):
        noc = ow // P
        secs.append({
            'ow': ow, 'T': T, 'noc': noc,
            'chunks': _token_chunks(T), 'wbs': _wblocks(ow),
            'xp': nc.dram_tensor(f"xp{si}", [P, 2 * KT * T], dt.float8e4,
                                 kind="ExternalInput"),
            'Whi': nc.dram_tensor(f"Whi{si}", [P, KT * ow],
                                  dt.float8e4, kind="ExternalInput"),
            'Wlo': nc.dram_tensor(f"Wlo{si}", [P, KT * ow],
                                  dt.float8e4, kind="ExternalInput"),
            'ia': nc.dram_tensor(f"ia{si}", [P, 1], dt.float32,
                                 kind="ExternalInput"),
            'yp': nc.dram_tensor(f"yp{si}", [P, noc * T], dt.bfloat16,
                                 kind="ExternalOutput"),
        })

    with TileContext(nc) as tc:
        with tc.tile_pool(name="persist", bufs=1) as pp, \
             tc.tile_pool(name="psum", bufs=8, space="PSUM") as ps:

            for si, s in enumerate(secs):
                s['xall'] = pp.tile([P, 2 * KT * s['T']], dt.float8e4,
                                    name=f"xall{si}", tag=f"xall{si}")
                s['whi'] = pp.tile([P, KT * s['ow']], dt.float8e4,
                                   name=f"whi{si}", tag=f"whi{si}")
                s['wlo'] = pp.tile([P, KT * s['ow']], dt.float8e4,
                                   name=f"wlo{si}", tag=f"wlo{si}")
                s['iat'] = pp.tile([P, 1], dt.float32,
                                   name=f"iat{si}", tag=f"iat{si}")
                s['ysb'] = pp.tile([P, s['noc'] * s['T']], dt.bfloat16,
                                   name=f"ysb{si}", tag=f"ysb{si}")

            # PE p-state warmup: a no-dep DoubleRow chain on a memset
            # tile keeps the clock ramping while the input DMAs fill.
            wm = pp.tile([P, 256], dt.float8e4, name="wm", tag="wm")
            nc.vector.memset(wm[:, :], 0)
            wmp = ps.tile([P, 128], dt.float32, name="wmp", tag="pt")
            w3 = wm[:, 0:256].rearrange("p (two c) -> p two c", two=2)
            for i in range(NWARM):
                nc.tensor.matmul(wmp[:, :], w3, w3, start=(i == 0),
                                 stop=(i == NWARM - 1),
                                 perf_mode=mybir.MatmulPerfMode.DoubleRow)

            for _rep in range(reps):
                # inputs in compute order on every queue: x first (pool),
                # W blocks per section in the order the PE consumes them
                # (W_hi on sync, W_lo + inva on scalar).
                for si_, s in enumerate(secs):
                    _emit_inputs(nc, s, si_ == 0)
                est = {'gi': 0}
                for si_, s in enumerate(secs):
                    _emit_compute(nc, dt, ps, s, si_ == len(secs) - 1,
                                  si_ >= len(secs) - 2, est)
    nc.compile()
    return nc


def _emit_inputs(nc, s, split_x):
    """Queue layout (each queue transfers at full rate in parallel):
      pool   : x_hi            (term 0 consumes it first)
      scalar : x_lo, W_lo blocks (terms 1-2 run ~0.4/0.8us later)
      sync   : W_hi b0, inva, W_hi rest (+ y flushes emitted later)
    The first section's x goes in two kt-pair chunks so term 0's j-loop
    starts on pair 0-1 while pair 2-3 is still in flight."""
    T = s['T']
    H = KT * T
    xp, xall = s['xp'], s['xall']
    xcuts = ((0, 4 * T), (4 * T, H)) if split_x else ((0, H),)
    for a, b in xcuts:
        nc.gpsimd.dma_start(xall[:, a:b], xp.ap()[:, a:b])
    (o0, bw, coloff) = s['wbs'][0]
    nc.sync.dma_start(s['whi'][:, coloff:coloff + KT * bw],
                      s['Whi'].ap()[:, coloff:coloff + KT * bw])
    nc.sync.dma_start(s['iat'][:, :], s['ia'].ap()[:, :])
    for a, b in xcuts:
        nc.scalar.dma_start(xall[:, H + a:H + b], xp.ap()[:, H + a:H + b])
    for (o0, bw, coloff) in s['wbs']:
        a, b = coloff, coloff + KT * bw
        nc.scalar.dma_start(s['wlo'][:, a:b], s['Wlo'].ap()[:, a:b])
    for (o0, bw, coloff) in s['wbs'][1:]:
        a, b = coloff, coloff + KT * bw
        nc.sync.dma_start(s['whi'][:, a:b], s['Whi'].ap()[:, a:b])


def _emit_compute(nc, dt, ps, s, last_sec, tail_sec, est):
    import concourse.mybir as mybir
    chunks, wbs = s['chunks'], s['wbs']
    ow, noc, T = s['ow'], s['noc'], s['T']
    xall, whi, wlo, iat, ysb = (s['xall'], s['whi'], s['wlo'], s['iat'],
                                s['ysb'])
    H = KT * T

    def wpair(wall, j, oc):
        # DoubleRow weights AP [128, 2, 128]: [:, i, :] = cols
        # [oc*128, oc*128+128) of kt=2j+i in the block layout
        for (o0, bw, coloff) in wbs:
            if o0 <= oc * P < o0 + bw:
                a = coloff + (2 * j) * bw
                s3 = wall[:, a:a + 2 * bw].rearrange(
                    "p (two c) -> p two c", two=2)
                off = oc * P - o0
                return s3[:, :, off:off + P]
        raise AssertionError

    def xpair(h, j, c0, clen):
        # DoubleRow ifmap AP [128, 2, clen]: [:, i, :] = xh/xl tokens
        # [c0, c0+clen) of kt=2j+i
        a = h * H + 2 * j * T
        s3 = xall[:, a:a + 2 * T].rearrange("p (two c) -> p two c", two=2)
        return s3[:, :, c0:c0 + clen]

    # oc-major emission: groups complete progressively behind the weight
    # stream; y DMAs stream on the gpsimd ring behind x.  A PSUM group
    # covers gsz consecutive ocs (1 for big-token sections) with one
    # merged evict.
    total = noc * T
    gsz = max(1, PSUM_FP32 // T) if len(chunks) == 1 else 1
    ypos = 0
    yprev = 0
    oc = 0
    while oc < noc:
        g = min(gsz, noc - oc)
        if last_sec and g > 1 and oc + g == noc:
            g -= 1          # leave a lone-oc final group: minimal tail
        glast = (oc + g == noc)
        occhunks = [(c0, clen, g) for (c0, clen) in chunks]
        if glast and g == 1 and len(chunks) == 1 and chunks[0][1] > 128:
            # split the final group into token halves: the first half's
            # evict+flush overlaps the second half's matmuls, shortening
            # the end-of-kernel serial tail
            fc0, fclen = chunks[0]
            hl = fclen // 2
            occhunks = [(fc0, hl, 1), (fc0 + hl, fclen - hl, 1)]
        for (c0, clen, gg) in occhunks:
            pt = ps.tile([P, gg * clen], dt.float32, name="pt", tag="pt")
            first = True
            for k in range(gg):
                for term in range(3):
                    wall = whi if term < 2 else wlo
                    h = 1 if term == 1 else 0
                    for j in range(NJ):
                        nc.tensor.matmul(
                            pt[:, k * clen:(k + 1) * clen],
                            wpair(wall, j, oc + k),
                            xpair(h, j, c0, clen),
                            start=first,
                            stop=(k == 0 and j == NJ - 1 and term == 2),
                            skip_group_check=(k > 0),
                            perf_mode=mybir.MatmulPerfMode.DoubleRow)
                        first = False
            ydst = ysb[:, oc * T + c0:oc * T + c0 + gg * clen]
            # trailing small sections: alternate evicts between ACT and
            # DVE so the end-of-kernel evict chains don't serialize on
            # one engine right behind the last matmuls
            is_final = (last_sec and glast and (c0, clen, gg) == occhunks[-1])
            if (tail_sec and T <= 128) or is_final:
                use_act = est.get('last_ev') != 'act'
            else:
                use_act = False
            if use_act:
                nc.scalar.activation(ydst, pt[:, :],
                                     mybir.ActivationFunctionType.Copy,
                                     scale=iat[:, 0:1])
                est['last_ev'] = 'act'
            else:
                nc.vector.tensor_scalar_mul(ydst, pt[:, :], iat[:, 0:1])
                est['last_ev'] = 'dve'
            ypos += gg * clen
        oc += g
        # flush y after 6 ocs, then every ~3.  Small last sections keep
        # everything for the single final sync flush (extra 500ns-min
        # pool DMAs at the end otherwise become the critical path).
        left = noc - oc
        if last_sec and total <= PSUM_FP32:
            do_flush = ypos == total
        else:
            do_flush = (ypos == total or (yprev == 0 and ypos >= 6 * T)
                        or (yprev > 0 and left > 2
                            and ypos - yprev >= 3 * T))
        if do_flush:
            q = nc.sync if (last_sec and ypos == total) else nc.gpsimd
            q.dma_start(s['yp'].ap()[:, yprev:ypos], ysb[:, yprev:ypos])
            yprev = ypos
    if yprev < total:
        nc.sync.dma_start(s['yp'].ap()[:, yprev:total], ysb[:, yprev:total])


# ---------------------------------------------------------------- host side

def _host_prep(input, tokens_per_expert, W_q, scales, zeros):
    x = np.asarray(input, dtype=np.float32)
    counts = np.asarray(tokens_per_expert, dtype=np.int32)
    Wq = np.asarray(W_q, dtype=np.int32)
    sc = np.asarray(scales, dtype=np.float32)
    zr = np.asarray(zeros, dtype=np.float32)

    Wf = (Wq.reshape(E, G, GS, OUT).astype(np.float32) - zr[:, :, None, :]) \
        * sc[:, :, None, :]
    W = Wf.reshape(E, IN, OUT)          # fp32; fp8 split happens per section
    xT = np.ascontiguousarray(x.T)      # [IN, N] fp32
    return xT, W, counts


def _core_specs(plan, counts, xT, W):
    import ml_dtypes
    f8 = ml_dtypes.float8_e4m3
    starts = np.concatenate([[0], np.cumsum(counts)]).astype(int)
    specs = []
    for sections in plan:
        key = []
        in_map = {}
        for si, p in enumerate(sections):
            e, ow, o0 = p['e'], p['ow'], p['o0']
            T = int(counts[e])
            key.append((T, ow))
            xsel = xT[:, starts[e]:starts[e] + T]      # [IN, T] fp32
            xh = xsel.astype(f8)
            xl = (xsel - xh.astype(np.float32)).astype(f8)
            xs = np.stack([xh, xl])                    # [2, IN, T]
            xp = np.ascontiguousarray(
                xs.reshape(2, KT, P, T).transpose(2, 0, 1, 3)
                .reshape(P, 2 * KT * T))
            ws = W[e][:, o0:o0 + ow]                   # [IN, ow]
            # per-PARTITION power-of-2 normalization (shared across the
            # section's ocs: column maxes concentrate, and a per-p alpha
            # lets multi-oc PSUM groups evict with one tensor_scalar)
            pmax = np.maximum(
                np.abs(ws).reshape(IN, ow // P, P).max(axis=(0, 1)), 1e-30)
            alpha_p = 2.0 ** np.floor(np.log2(224.0 / pmax))   # [P]
            alpha = np.tile(alpha_p[None, :], (ow // P, 1)).reshape(ow)
            wn = ws * alpha[None, :]
            wh = wn.astype(f8)
            wl = (wn - wh.astype(np.float32)).astype(f8)
            blocks_h, blocks_l = [], []
            for arr, blocks in ((wh, blocks_h), (wl, blocks_l)):
                wkpo = arr.reshape(KT, P, ow)
                for (b0, bw, coloff) in _wblocks(ow):
                    blk = wkpo[:, :, b0:b0 + bw]       # [kt, p, bw]
                    blocks.append(blk.transpose(1, 0, 2).reshape(P, -1))
            ia = (1.0 / alpha_p)[:, None]              # [P, 1]
            in_map[f"xp{si}"] = xp
            in_map[f"Whi{si}"] = np.ascontiguousarray(
                np.concatenate(blocks_h, axis=1))
            in_map[f"Wlo{si}"] = np.ascontiguousarray(
                np.concatenate(blocks_l, axis=1))
            in_map[f"ia{si}"] = np.ascontiguousarray(ia.astype(np.float32))
        specs.append({'key': tuple(key), 'in_map': in_map,
                      'plan': sections})
    return specs


def _assemble(plan, counts, results):
    starts = np.concatenate([[0], np.cumsum(counts)]).astype(int)
    y = np.empty((N, OUT), np.float32)
    for sections, res in zip(plan, results):
        for si, p in enumerate(sections):
            e, ow, o0 = p['e'], p['ow'], p['o0']
            noc = ow // P
            T = int(counts[e])
            yp = np.asarray(res[f"yp{si}"]).astype(np.float32)
            yT = yp.reshape(P, noc, T).transpose(1, 0, 2).reshape(ow, T)
            y[starts[e]:starts[e] + T, o0:o0 + ow] = yT.T
    return y


# ---------------------------------------------------------------- runner

def _io_spec(nc):
    import concourse.mybir as mybir
    import jax
    in_names, out_names, out_avals, zero_outs = [], [], [], []
    for alloc in nc.m.functions[0].allocations:
        if not isinstance(alloc, mybir.MemoryLocationSet):
            continue
        name = alloc.memorylocations[0].name
        if alloc.kind == "ExternalInput":
            in_names.append(name)
        elif alloc.kind == "ExternalOutput":
            out_names.append(name)
            shape = tuple(alloc.tensor_shape)
            dtype = mybir.dt.np(alloc.dtype)
            out_avals.append(jax.core.ShapedArray(shape, dtype))
            zero_outs.append(np.zeros(shape, dtype))
    return in_names, out_names, out_avals, zero_outs


def _make_jit(nc):
    import jax
    from concourse import bass2jax
    in_names, out_names, out_avals, zero_outs = _io_spec(nc)
    assert nc.dbg_addr is None
    partition_name = (nc.partition_id_tensor.name
                      if nc.partition_id_tensor else None)
    in_names = [n for n in in_names if n != partition_name]
    nparams = len(in_names)
    all_names = in_names + out_names
    if partition_name is not None:
        all_names = all_names + [partition_name]

    def _body(*args):
        operands = list(args)
        if partition_name is not None:
            operands.append(bass2jax.partition_id_tensor())
        outs = bass2jax._bass_exec_p.bind(
            *operands,
            out_avals=tuple(out_avals),
            in_names=tuple(all_names),
            out_names=tuple(out_names),
            lowering_input_output_aliases=(),
            sim_require_finite=True,
            sim_require_nnan=True,
            nc=nc,
        )
        return tuple(outs)

    donate = tuple(range(nparams, nparams + len(out_avals)))
    return (jax.jit(_body, donate_argnums=donate, keep_unused=True),
            in_names, out_names, zero_outs)


def _run_multi(specs, ncs):
    import jax
    from concourse import bass2jax
    bass2jax.install_neuronx_cc_hook()
    devices = jax.devices()
    fns = {key: _make_jit(nc) for key, nc in ncs.items()}
    pending = []
    for i, spec in enumerate(specs):
        fn, in_names, out_names, zero_outs = fns[spec['key']]
        dev = devices[i]
        args = [jax.device_put(np.asarray(spec['in_map'][n]), dev)
                for n in in_names]
        args += [jax.device_put(z, dev) for z in zero_outs]
        pending.append((fn(*args), out_names))
    return [{nm: np.asarray(o) for nm, o in zip(out_names, outs)}
            for outs, out_names in pending]


# ---------------------------------------------------------------- entry

def kernel(input, tokens_per_expert, W_q, scales, zeros):
    xT, W, counts = _host_prep(input, tokens_per_expert, W_q, scales, zeros)
    plan = _make_plan(counts)
    specs = _core_specs(plan, counts, xT, W)
    ncs = {}
    for spec in specs:
        key = spec['key']
        if key not in ncs:
            ncs[key] = _build_prog(key)
    results = _run_multi(specs, ncs)
    return _assemble(plan, counts, results)
